# revision 32
# baseline (speedup 1.0000x reference)
"""Multi-head attention (B=2, S=2048, E=1024, H=16, D=64) on 8 TRN2 NeuronCores.

Sharding: tensor-parallel over heads (2 heads/core) for QKV projections and
attention; on-device AllToAlls reshard the attention output so each core
owns 512 rows; row-parallel output projection; host concatenates the row
slices. Inputs are host-cast to bf16 and x is host-transposed (the
contraction dim must sit on SBUF partitions); all matmul accumulation is
fp32 on-chip.

Scheduling is a fine-grained software pipeline built to keep the PE's HAM
clock-gate warm (no idle gap near the ~3.4us MID window): per 128-key tick,
both heads' score matmuls (disjoint 64-row groups, concurrent in the PE)
fill one 2-bank PSUM tile, one N=1024 ACT exp evicts it to bf16, the PV
matmuls of tick-2 ride behind, and a filler iterator weaves the remaining
projections, V transposes and the first half of the output projection
through the PE slack. Units are q-REASSIGNED: units 0-3 cover the first
256-row half of every core's row slice, units 4-7 the second half, so the
AllToAll splits in two - A2A#1 launches mid-attention and is fully hidden,
and only A2A#2 (0.5 MB) is exposed at the tail. Softmax: a ones-column on
V accumulates the denominator inside PV; the pv PSUM is copied to SBUF
immediately (frees the bank), the reciprocal uses the fast custom-DVE
approximation, and GPSIMD broadcasts it for the DVE normalize. Dummy
matmuls cover the initial DMA ramp and the A2A#2 window; dummy AllReduces
drain launch skew from the CC queue.
"""

import sys

if "/opt/trn_rl_repo" not in sys.path:
    sys.path.insert(0, "/opt/trn_rl_repo")

from contextlib import ExitStack

import numpy as np

import concourse.bacc as bacc
import concourse.mybir as mybir
import concourse.tile as tile
from concourse.masks import make_identity

F32 = mybir.dt.float32
BF16 = mybir.dt.bfloat16
AF = mybir.ActivationFunctionType

_CACHE = {}


def build_kernel(B=2, S=2048, E=1024, H=16, D=64, N_CORES=8):
    HL = H // N_CORES
    HIDL = HL * D
    R = B * S
    RL = R // N_CORES
    EC = E // 128
    S128 = S // 128
    QB = 512
    HB = QB // 2  # 256-row half-blocks moved by each A2A
    NQB = S // QB
    RT = R // 128
    NG = R // QB
    NT = NG * S128  # total attention ticks (one per (unit, key-chunk))
    assert HIDL == 128 and D == 64 and QB == RL
    assert NG == N_CORES and S % QB == 0

    # q-reassignment: unit u covers two 256-row half-slices (batch, s0_a,
    # s0_b); units 0-3 hit the FIRST half of every core's 512-row slice
    # (cores 2u, 2u+1), units 4-7 the second half.
    UNIT_MAP = [(0, 0, 512), (0, 1024, 1536), (1, 0, 512), (1, 1024, 1536),
                (0, 256, 768), (0, 1280, 1792), (1, 256, 768),
                (1, 1280, 1792)]

    nc = bacc.Bacc("TRN2", target_bir_lowering=False, debug=False,
                   num_devices=N_CORES)

    # wqkv: host-packed [128, (w,i) blocks of 128] so DRAM rows are 6KB
    # contiguous (one DMA, big packets); bqkv: biases as three 512B rows
    xt_d = nc.dram_tensor("xt", [E, R], BF16, kind="ExternalInput")
    wqkv_d = nc.dram_tensor("wqkv", [128, 3 * EC * HIDL], BF16,
                            kind="ExternalInput")
    wo_d = nc.dram_tensor("wo", [E, E], BF16, kind="ExternalInput")
    bqkv_d = nc.dram_tensor("bqkv", [3, HIDL], F32, kind="ExternalInput")
    bo_d = nc.dram_tensor("bo", [1, E], BF16, kind="ExternalInput")
    out_d = nc.dram_tensor("out", [RL, E], F32, kind="ExternalOutput")

    with tile.TileContext(nc) as tc, ExitStack() as ctx:
        const = ctx.enter_context(tc.tile_pool(name="const", bufs=1))
        big = ctx.enter_context(tc.tile_pool(name="big", bufs=1))
        stage = ctx.enter_context(tc.tile_pool(name="stage", bufs=4))
        dram = ctx.enter_context(tc.tile_pool(name="dram", bufs=1, space="DRAM"))

        # dummy collective #1: absorbs cross-core launch skew on the CC queue
        sync_sb = const.tile([1, 512], F32)
        nc.vector.memset(sync_sb, 1.0)
        sync_in = dram.tile([1, 512], F32)
        sync_out = dram.tile([1, 512], F32)
        nc.sync.dma_start(out=sync_in[:], in_=sync_sb[:])
        nc.gpsimd.collective_compute(
            "AllReduce", mybir.AluOpType.add,
            replica_groups=[list(range(N_CORES))],
            ins=[sync_in.opt()], outs=[sync_out.opt()])

        # ---- constants / small weights (sync queue, before xT) ----
        ident = const.tile([128, 128], BF16)
        make_identity(nc, ident)
        ones_st = const.tile([1, 128], BF16)
        nc.vector.memset(ones_st, 1.0)
        wqkv_sb = const.tile([128, 3 * EC, HIDL], BF16)
        nc.sync.dma_start(out=wqkv_sb[:], in_=wqkv_d[:])
        b_row = const.tile([3, HIDL], F32)
        nc.sync.dma_start(out=b_row[:], in_=bqkv_d[:])
        w_tiles = {}
        for wi, wname in enumerate(("wq", "wk", "wv")):
            for i in range(EC):
                w_tiles[(wname, i)] = wqkv_sb[:, EC * wi + i, :]

        # ---- x^T loads: q-block-pair major so projections start early ----
        xT = big.tile([128, EC, R], BF16)
        for rp2 in range(NG // 2):
            c0, c1 = 1024 * rp2, 1024 * (rp2 + 1)
            for i in range(EC):
                eng = nc.scalar if i % 2 == 0 else nc.sync
                # two partition-half DMAs per chunk engage more HW queues
                for p in (0, 64):
                    eng.dma_start(
                        out=xT[p:p + 64, i, c0:c1],
                        in_=xt_d[128 * i + p:128 * i + p + 64, c0:c1])

        # wo / bo needed only at the end; scalar queue, after xT
        bo_sb = const.tile([1, E], BF16)
        nc.scalar.dma_start(out=bo_sb[:], in_=bo_d[:])
        wo_tiles = []
        for i in range(EC):
            t = const.tile([128, E], BF16, name=f"wo_{i}")
            nc.scalar.dma_start(out=t[:], in_=wo_d[128 * i:128 * (i + 1), :])
            wo_tiles.append(t)

        # biases arrive as rows; one PE transpose puts them per-partition
        ident32 = const.tile([128, 128], F32)
        make_identity(nc, ident32)
        b_sb = const.tile([128, 3], F32)
        b_tiles = {}

        # QT is [128, 256-col blocks, 256] so a unit's two half-slices form
        # one strided moving operand (single score matmul per head per tick)
        QT = big.tile([128, R // 256, 256], BF16)
        KT = big.tile([128, R], BF16)
        VT = big.tile([128, R], BF16)
        Vext = big.tile([128, HL, RT, D + 1], BF16)
        # softmax-denominator ones column, written once
        for h in range(HL):
            nc.vector.memset(Vext[:, h, :, D:D + 1], 1.0)

        # PSUM budget (8 banks): sc 2x2 + fill 2x1 + pv0 1 + pv1 1
        att_stack = ExitStack()
        att_psum = att_stack.enter_context(
            tc.tile_pool(name="att_psum", bufs=2, space="PSUM"))
        ebp = ctx.enter_context(tc.tile_pool(name="ebp", bufs=6))
        rp = ctx.enter_context(tc.tile_pool(name="rp", bufs=2))

        # PE warmth filler: dense matmuls with no real consumers. One byte is
        # DMA'd out at the end so DCE keeps the chain.
        wup_sink = dram.tile([1, 4], BF16)
        wup_sb = const.tile([1, 4], BF16)

        def warmup(n, mov, reps, flush=False):
            for _ in range(n):
                wps = att_psum.tile([128, 2, QB], F32, tag="sc", bufs=2,
                                    name="wps")
                nf = mov.shape[-1]
                for w in range(reps):
                    nc.tensor.matmul(wps[:, 0, 0:nf], ident[:], mov,
                                     start=(w == 0), stop=(w == reps - 1))
                nc.vector.tensor_copy(out=wup_sb[:], in_=wps[0:1, 0, 0:4])
            if flush:
                nc.sync.dma_start(out=wup_sink[:], in_=wup_sb[:])

        # ---- projection / V-transpose generators (yield ~0.4us PE quanta) --
        def proj_quanta(wname, bname, out_t, rb):
            ps = att_psum.tile([128, QB], F32, tag="fill", bufs=2,
                               name="fill_ps")
            for i in range(EC):
                nc.tensor.matmul(ps[:], w_tiles[(wname, i)][:],
                                 xT[:, i, QB * rb:QB * (rb + 1)],
                                 start=(i == 0), stop=(i == EC - 1))
                if i % 2 == 1 and i < EC - 1:
                    yield
            if out_t is QT:
                out_ap = QT[:, 2 * rb:2 * rb + 2, :]
            else:
                out_ap = out_t[:, QB * rb:QB * (rb + 1)]
            nc.vector.tensor_scalar_add(
                out=out_ap, in0=ps[:], scalar1=b_tiles[bname][:])
            yield

        def vext_quanta(kt):
            vps = att_psum.tile([128, 128], BF16, tag="fill", bufs=2,
                                name="vtr_ps")
            nc.tensor.transpose(vps[:], VT[:, 128 * kt:128 * (kt + 1)],
                                ident[:])
            for h in range(HL):
                nc.vector.tensor_copy(out=Vext[:, h, kt, 0:D],
                                      in_=vps[:, D * h:D * (h + 1)])
            yield

        def run_all(gen):
            for _ in gen:
                pass

        # ---- pre-phase: only the projections gated on the FIRST x^T
        # chunk-pair (b0 q-cols 0:1024); warm-drip dummies span the DMA ramp
        # so the HAM clock-gate stays open. Everything else weaves through
        # the attention ticks as deadline-ordered filler. ----
        bps = att_psum.tile([128, 3], F32, tag="fill", bufs=2, name="bps")
        nc.tensor.transpose(bps[:], b_row[:], ident32[0:3, 0:3])
        nc.vector.tensor_copy(out=b_sb[:], in_=bps[:])
        for j, bname in enumerate(("bq", "bk", "bv")):
            b_tiles[bname] = b_sb[:, j:j + 1]
        warmup(8, ident[:, 0:128], 8)
        run_all(proj_quanta("wk", "bk", KT, 0))
        warmup(2, ident[:, 0:128], 8)
        run_all(proj_quanta("wk", "bk", KT, 1))
        warmup(2, ident[:, 0:128], 8)
        run_all(proj_quanta("wq", "bq", QT, 0))
        run_all(proj_quanta("wq", "bq", QT, 1))
        for rb in (0, 1):
            run_all(proj_quanta("wv", "bv", VT, rb))
            for kt in range(4 * rb, 4 * rb + 4):
                run_all(vext_quanta(kt))
        warmup(0, ident[:, 0:128], 8, flush=True)

        # dummy collective #2: re-sync the CC queue before the attention phase
        sync2_in = dram.tile([1, 256], BF16)
        sync2_out = dram.tile([1, 256], BF16)
        nc.sync.dma_start(out=sync2_in[:], in_=VT[0:1, 1792:2048])
        nc.gpsimd.collective_compute(
            "AllReduce", mybir.AluOpType.add,
            replica_groups=[list(range(N_CORES))],
            ins=[sync2_in.opt()], outs=[sync2_out.opt()])

        # ---- attention: fine-grained tick pipeline ----
        a2a1_in = dram.tile([NG * HIDL, HB], BF16)
        a2a1_out = dram.tile([NG * HIDL, HB], BF16)
        a2a2_in = dram.tile([NG * HIDL, HB], BF16)
        a2a2_out = dram.tile([NG * HIDL, HB], BF16)
        ATn = big.tile([128, NG, QB], BF16)
        AT1 = big.tile([128, EC, HB], BF16)
        AT2 = big.tile([128, EC, HB], BF16)

        eb = {}
        pvT = {}

        def emit_scores(t):
            u, j = divmod(t, S128)
            b, s0a, s0b = UNIT_MAP[u]
            X = att_psum.tile([128, 2, QB], F32, tag="sc", bufs=2,
                              name="sc_ps")
            blk = (b * S + s0a) // HB
            for h in range(HL):
                hs = slice(64 * h, 64 * (h + 1))
                nc.tensor.matmul(
                    X[:, h, :],
                    KT[hs, b * S + 128 * j:b * S + 128 * (j + 1)],
                    QT[hs, blk:blk + 3:2, :],
                    start=True, stop=True)
            e = ebp.tile([128, 2, QB], BF16, tag="eb", bufs=6, name="eb")
            nc.scalar.activation(e[:], X[:], AF.Exp, scale=0.125)
            eb[t] = e

        def emit_pv(t):
            u, j = divmod(t, S128)
            b = UNIT_MAP[u][0]
            for h in range(HL):
                if j == 0:
                    pvT[(u, h)] = att_psum.tile(
                        [D + 1, QB], F32, tag=f"pv{h}", bufs=1,
                        name=f"pv{h}_ps")
                nc.tensor.matmul(pvT[(u, h)][:],
                                 Vext[:, h, b * S128 + j, :],
                                 eb[t][:, h, :],
                                 start=(j == 0), stop=(j == S128 - 1))
            del eb[t]

        def emit_norm(u):
            for h in range(HL):
                hs = slice(64 * h, 64 * (h + 1))
                pvsb = rp.tile([D + 1, QB], F32, tag=f"pvsb{h}", bufs=2,
                               name=f"pvsb{h}")
                nc.vector.tensor_copy(out=pvsb[:], in_=pvT[(u, h)][:])
                den = rp.tile([1, QB], F32, tag=f"den{h}", bufs=2,
                              name=f"den{h}")
                nc.vector.tensor_copy(out=den[:], in_=pvT[(u, h)][D:D + 1, :])
                r_row = rp.tile([1, QB], F32, tag=f"rr{h}", bufs=2,
                                name=f"rr{h}")
                nc.vector.reciprocal_approx_fast(r_row[:], den[:])
                r_sb = rp.tile([D, QB], F32, tag=f"rb{h}", bufs=2,
                               name=f"rb{h}")
                nc.gpsimd.partition_broadcast(r_sb[:], r_row[:])
                nc.vector.tensor_mul(
                    out=ATn[hs, u, :], in0=pvsb[0:D, :], in1=r_sb[:])
            # unit u's two 256-col halves are shards 2u', 2u'+1 of its A2A
            a_in = a2a1_in if u < 4 else a2a2_in
            ushard = 2 * (u % 4)
            for k in range(2):
                nc.sync.dma_start(
                    out=a_in[HIDL * (ushard + k):HIDL * (ushard + k + 1), :],
                    in_=ATn[:, u, HB * k:HB * (k + 1)])

        def oproj_quanta(qq, AT, o_rows):
            """Output projection for one 128-row block (quantum generator)."""
            o_sb = stage.tile([128, E], F32, tag="osb", bufs=2, name="osb")
            for e_c in range(E // QB):
                ps = att_psum.tile([128, QB], F32, tag="fill", bufs=2,
                                   name="op_ps")
                nc.tensor.matmul(ps[:], ones_st[:],
                                 bo_sb[:, QB * e_c:QB * (e_c + 1)],
                                 start=True, stop=False)
                for i in range(EC):
                    nc.tensor.matmul(ps[:], AT[:, i, 128 * qq:128 * (qq + 1)],
                                     wo_tiles[i][:, QB * e_c:QB * (e_c + 1)],
                                     start=False, stop=(i == EC - 1))
                    if i % 3 == 2:
                        yield
                nc.vector.tensor_copy(out=o_sb[:, QB * e_c:QB * (e_c + 1)],
                                      in_=ps[:])
                eng = nc.sync if e_c == 0 else nc.scalar
                eng.dma_start(
                    out=out_d[o_rows:o_rows + 128, QB * e_c:QB * (e_c + 1)],
                    in_=o_sb[:, QB * e_c:QB * (e_c + 1)])
                yield

        def filler_gen():
            # deadline-ordered: (rest of b0 for units 0-1) then b1 staged to
            # land just before units 2-3 consume each piece
            yield from proj_quanta("wk", "bk", KT, 2)
            yield from proj_quanta("wk", "bk", KT, 3)
            yield from proj_quanta("wv", "bv", VT, 2)
            for kt in range(8, 12):
                yield from vext_quanta(kt)
            yield from proj_quanta("wv", "bv", VT, 3)
            for kt in range(12, 16):
                yield from vext_quanta(kt)
            yield from proj_quanta("wq", "bq", QT, 2)
            yield from proj_quanta("wq", "bq", QT, 3)
            yield from proj_quanta("wk", "bk", KT, NQB)
            yield from proj_quanta("wq", "bq", QT, NQB)
            yield from proj_quanta("wq", "bq", QT, NQB + 1)
            yield from proj_quanta("wv", "bv", VT, NQB)
            for kt in range(16, 20):
                yield from vext_quanta(kt)
            yield from proj_quanta("wk", "bk", KT, NQB + 1)
            yield from proj_quanta("wv", "bv", VT, NQB + 1)
            for kt in range(20, 24):
                yield from vext_quanta(kt)
            yield from proj_quanta("wk", "bk", KT, NQB + 2)
            yield from proj_quanta("wv", "bv", VT, NQB + 2)
            for kt in range(24, 28):
                yield from vext_quanta(kt)
            yield from proj_quanta("wk", "bk", KT, NQB + 3)
            yield from proj_quanta("wq", "bq", QT, NQB + 2)
            yield from proj_quanta("wq", "bq", QT, NQB + 3)
            yield from proj_quanta("wv", "bv", VT, NQB + 3)
            for kt in range(28, 32):
                yield from vext_quanta(kt)

        def oproj1_gen():
            # first-half output projection; only consumed well after A2A#1
            # has landed (its matmuls would otherwise block the in-order PE
            # queue on the collective)
            yield from oproj_quanta(0, AT1, 0)
            yield from oproj_quanta(1, AT1, 128)

        fill = filler_gen()
        fill2 = oproj1_gen()
        fills_left = True
        fills2_left = True
        for t in range(NT + 2):
            if t < NT:
                emit_scores(t)
            if t >= 2:
                emit_pv(t - 2)
                u_done, j_done = divmod(t - 2, S128)
                if j_done == S128 - 1:
                    emit_norm(u_done)
                    if u_done == 3:
                        # first-half shards complete: launch hidden A2A#1
                        nc.gpsimd.collective_compute(
                            "AllToAll", mybir.AluOpType.bypass,
                            replica_groups=[list(range(N_CORES))],
                            ins=[a2a1_in.opt()], outs=[a2a1_out.opt()])
                        # scalar queue: idle by now, and an AT1 unload on the
                        # sync queue would block later a2a2_in stores behind
                        # the collective wait
                        for i in range(N_CORES):
                            nc.scalar.dma_start(
                                out=AT1[:, i, :],
                                in_=a2a1_out[HIDL * i:HIDL * (i + 1), :])
            n_q = 3 if t < 16 else (2 if t < 56 else 1)
            for _ in range(n_q):
                if fills_left:
                    try:
                        next(fill)
                    except StopIteration:
                        fills_left = False

        nc.gpsimd.collective_compute(
            "AllToAll", mybir.AluOpType.bypass,
            replica_groups=[list(range(N_CORES))],
            ins=[a2a2_in.opt()], outs=[a2a2_out.opt()])
        for i in range(N_CORES):
            nc.scalar.dma_start(out=AT2[:, i, :],
                              in_=a2a2_out[HIDL * i:HIDL * (i + 1), :])

        # drain leftover fillers; the first-half out projection is real PE
        # work that covers the A2A#2 flight (AT1 landed long ago)
        while fills_left:
            try:
                next(fill)
            except StopIteration:
                fills_left = False
        while fills2_left:
            try:
                next(fill2)
            except StopIteration:
                fills2_left = False
        warmup(8, ATn[:, NG - 1, 0:QB], 4, flush=True)

        # ---- second-half out projection ----
        run_all(oproj_quanta(0, AT2, 256))
        run_all(oproj_quanta(1, AT2, 384))
        att_stack.close()

    nc.compile()
    return nc


def shard_inputs(x, Wq, bq, Wk, bk, Wv, bv, Wo, bo, N_CORES=8):
    """Host-side sharding: full fp32 inputs -> per-core in_maps."""
    import ml_dtypes
    bf16 = ml_dtypes.bfloat16
    B, S, E = x.shape
    R = B * S
    HIDL = E // N_CORES
    xt = np.ascontiguousarray(x.reshape(R, E).T).astype(bf16)
    wo = np.ascontiguousarray(Wo).astype(bf16)
    bo_b = np.ascontiguousarray(bo.reshape(1, E)).astype(bf16)
    EC = E // 128
    in_maps = []
    for c in range(N_CORES):
        cs = slice(HIDL * c, HIDL * (c + 1))
        # pack wq/wk/wv E-chunks side by side: rows are 6KB contiguous
        wqkv = np.empty((128, 3 * EC * HIDL), dtype=bf16)
        for wi, W in enumerate((Wq, Wk, Wv)):
            Wc = W[:, cs]
            for i in range(EC):
                blk = EC * wi + i
                wqkv[:, HIDL * blk:HIDL * (blk + 1)] = Wc[128 * i:128 * (i + 1), :]
        bqkv = np.stack([bq[cs], bk[cs], bv[cs]]).astype(np.float32)
        in_maps.append({
            "xt": xt,
            "wqkv": wqkv,
            "wo": wo,
            "bqkv": np.ascontiguousarray(bqkv),
            "bo": bo_b,
        })
    return in_maps


def kernel(x, Wq, bq, Wk, bk, Wv, bv, Wo, bo):
    from concourse.bass_utils import run_bass_kernel_spmd

    args = [np.asarray(a, dtype=np.float32) for a in
            (x, Wq, bq, Wk, bk, Wv, bv, Wo, bo)]
    if "nc" not in _CACHE:
        _CACHE["nc"] = build_kernel()
    nc = _CACHE["nc"]
    in_maps = shard_inputs(*args)
    res = run_bass_kernel_spmd(nc, in_maps, core_ids=list(range(8)))
    out = np.concatenate([res.results[i]["out"] for i in range(8)], axis=0)
    return out.reshape(2, 2048, 1024)


# revision 35
# speedup vs baseline: 1.0053x; 1.0053x over previous
"""Multi-head attention (B=2, S=2048, E=1024, H=16, D=64) on 8 TRN2 NeuronCores.

Sharding: tensor-parallel over heads (2 heads/core) for QKV projections and
attention; on-device AllToAlls reshard the attention output so each core
owns 512 rows; row-parallel output projection; host concatenates the row
slices. Inputs are host-cast to bf16 and x is host-transposed (the
contraction dim must sit on SBUF partitions); all matmul accumulation is
fp32 on-chip.

Scheduling is a fine-grained software pipeline built to keep the PE's HAM
clock-gate warm (no idle gap near the ~3.4us MID window): per 128-key tick,
both heads' score matmuls (disjoint 64-row groups, concurrent in the PE)
fill one 2-bank PSUM tile, one N=1024 ACT exp evicts it to bf16, the PV
matmuls of tick-2 ride behind, and a filler iterator weaves the remaining
projections, V transposes and the first half of the output projection
through the PE slack. Units are q-REASSIGNED: units 0-3 cover the first
256-row half of every core's row slice, units 4-7 the second half, so the
AllToAll splits in two - A2A#1 launches mid-attention and is fully hidden,
and only A2A#2 (0.5 MB) is exposed at the tail. Softmax: a ones-column on
V accumulates the denominator inside PV; the pv PSUM is copied to SBUF
immediately (frees the bank), the reciprocal uses the fast custom-DVE
approximation, and GPSIMD broadcasts it for the DVE normalize. Dummy
matmuls cover the initial DMA ramp and the A2A#2 window; dummy AllReduces
drain launch skew from the CC queue.
"""

import sys

if "/opt/trn_rl_repo" not in sys.path:
    sys.path.insert(0, "/opt/trn_rl_repo")

from contextlib import ExitStack

import numpy as np

import concourse.bacc as bacc
import concourse.mybir as mybir
import concourse.tile as tile
from concourse.masks import make_identity

F32 = mybir.dt.float32
BF16 = mybir.dt.bfloat16
AF = mybir.ActivationFunctionType

_CACHE = {}


def build_kernel(B=2, S=2048, E=1024, H=16, D=64, N_CORES=8):
    HL = H // N_CORES
    HIDL = HL * D
    R = B * S
    RL = R // N_CORES
    EC = E // 128
    S128 = S // 128
    QB = 512
    HB = QB // 2  # 256-row half-blocks moved by each A2A
    NQB = S // QB
    RT = R // 128
    NG = R // QB
    NT = NG * S128  # total attention ticks (one per (unit, key-chunk))
    assert HIDL == 128 and D == 64 and QB == RL
    assert NG == N_CORES and S % QB == 0

    # q-reassignment: unit u covers two 256-row half-slices (batch, s0_a,
    # s0_b); units 0-3 hit the FIRST half of every core's 512-row slice
    # (cores 2u, 2u+1), units 4-7 the second half.
    UNIT_MAP = [(0, 0, 512), (0, 1024, 1536), (1, 0, 512), (1, 1024, 1536),
                (0, 256, 768), (0, 1280, 1792), (1, 256, 768),
                (1, 1280, 1792)]

    nc = bacc.Bacc("TRN2", target_bir_lowering=False, debug=False,
                   num_devices=N_CORES)

    # wqkv: host-packed [128, (w,i) blocks of 128] so DRAM rows are 6KB
    # contiguous (one DMA, big packets); bqkv: biases as three 512B rows
    xt_d = nc.dram_tensor("xt", [E, R], BF16, kind="ExternalInput")
    wqkv_d = nc.dram_tensor("wqkv", [128, 3 * EC * HIDL], BF16,
                            kind="ExternalInput")
    wo_d = nc.dram_tensor("wo", [E, E], BF16, kind="ExternalInput")
    bqkv_d = nc.dram_tensor("bqkv", [3, HIDL], F32, kind="ExternalInput")
    bo_d = nc.dram_tensor("bo", [1, E], BF16, kind="ExternalInput")
    out_d = nc.dram_tensor("out", [RL, E], F32, kind="ExternalOutput")

    with tile.TileContext(nc) as tc, ExitStack() as ctx:
        const = ctx.enter_context(tc.tile_pool(name="const", bufs=1))
        big = ctx.enter_context(tc.tile_pool(name="big", bufs=1))
        stage = ctx.enter_context(tc.tile_pool(name="stage", bufs=4))
        dram = ctx.enter_context(tc.tile_pool(name="dram", bufs=1, space="DRAM"))

        # dummy collective #1: absorbs cross-core launch skew on the CC queue
        sync_sb = const.tile([1, 512], F32)
        nc.vector.memset(sync_sb, 1.0)
        sync_in = dram.tile([1, 512], F32)
        sync_out = dram.tile([1, 512], F32)
        nc.sync.dma_start(out=sync_in[:], in_=sync_sb[:])
        nc.gpsimd.collective_compute(
            "AllReduce", mybir.AluOpType.add,
            replica_groups=[list(range(N_CORES))],
            ins=[sync_in.opt()], outs=[sync_out.opt()])

        # ---- constants / small weights (sync queue, before xT) ----
        ident = const.tile([128, 128], BF16)
        make_identity(nc, ident)
        ones_st = const.tile([1, 128], BF16)
        nc.vector.memset(ones_st, 1.0)
        wqkv_sb = const.tile([128, 3 * EC, HIDL], BF16)
        nc.sync.dma_start(out=wqkv_sb[:], in_=wqkv_d[:])
        b_row = const.tile([3, HIDL], F32)
        nc.sync.dma_start(out=b_row[:], in_=bqkv_d[:])
        w_tiles = {}
        for wi, wname in enumerate(("wq", "wk", "wv")):
            for i in range(EC):
                w_tiles[(wname, i)] = wqkv_sb[:, EC * wi + i, :]

        # ---- x^T loads: q-block-pair major so projections start early ----
        xT = big.tile([128, EC, R], BF16)
        for rp2 in range(NG // 2):
            c0, c1 = 1024 * rp2, 1024 * (rp2 + 1)
            for i in range(EC):
                eng = nc.scalar if i % 2 == 0 else nc.sync
                # two partition-half DMAs per chunk engage more HW queues
                for p in (0, 64):
                    eng.dma_start(
                        out=xT[p:p + 64, i, c0:c1],
                        in_=xt_d[128 * i + p:128 * i + p + 64, c0:c1])

        # wo / bo needed only at the end; scalar queue, after xT
        bo_sb = const.tile([1, E], BF16)
        nc.scalar.dma_start(out=bo_sb[:], in_=bo_d[:])
        wo_tiles = []
        for i in range(EC):
            t = const.tile([128, E], BF16, name=f"wo_{i}")
            nc.scalar.dma_start(out=t[:], in_=wo_d[128 * i:128 * (i + 1), :])
            wo_tiles.append(t)

        # biases arrive as rows; one PE transpose puts them per-partition
        ident32 = const.tile([128, 128], F32)
        make_identity(nc, ident32)
        b_sb = const.tile([128, 3], F32)
        b_tiles = {}

        # QT is [128, 256-col blocks, 256] so a unit's two half-slices form
        # one strided moving operand (single score matmul per head per tick)
        QT = big.tile([128, R // 256, 256], BF16)
        KT = big.tile([128, R], BF16)
        VT = big.tile([128, R], BF16)
        Vext = big.tile([128, HL, RT, D + 1], BF16)
        # softmax-denominator ones column, written once
        for h in range(HL):
            nc.vector.memset(Vext[:, h, :, D:D + 1], 1.0)

        # PSUM budget (8 banks): sc 2x2 + fill 2x1 + pv0 1 + pv1 1
        att_stack = ExitStack()
        att_psum = att_stack.enter_context(
            tc.tile_pool(name="att_psum", bufs=2, space="PSUM"))
        ebp = ctx.enter_context(tc.tile_pool(name="ebp", bufs=6))
        rp = ctx.enter_context(tc.tile_pool(name="rp", bufs=2))

        # PE warmth filler: dense matmuls with no real consumers. One byte is
        # DMA'd out at the end so DCE keeps the chain.
        wup_sink = dram.tile([1, 4], BF16)
        wup_sb = const.tile([1, 4], BF16)

        def warmup(n, mov, reps, flush=False):
            for _ in range(n):
                wps = att_psum.tile([128, 2, QB], F32, tag="sc", bufs=2,
                                    name="wps")
                nf = mov.shape[-1]
                for w in range(reps):
                    nc.tensor.matmul(wps[:, 0, 0:nf], ident[:], mov,
                                     start=(w == 0), stop=(w == reps - 1))
                nc.vector.tensor_copy(out=wup_sb[:], in_=wps[0:1, 0, 0:4])
            if flush:
                nc.sync.dma_start(out=wup_sink[:], in_=wup_sb[:])

        # ---- projection / V-transpose generators (yield ~0.4us PE quanta) --
        def proj_quanta(wname, bname, out_t, rb):
            ps = att_psum.tile([128, QB], F32, tag="fill", bufs=2,
                               name="fill_ps")
            for i in range(EC):
                nc.tensor.matmul(ps[:], w_tiles[(wname, i)][:],
                                 xT[:, i, QB * rb:QB * (rb + 1)],
                                 start=(i == 0), stop=(i == EC - 1))
                if i % 2 == 1 and i < EC - 1:
                    yield
            if out_t is QT:
                out_ap = QT[:, 2 * rb:2 * rb + 2, :]
            else:
                out_ap = out_t[:, QB * rb:QB * (rb + 1)]
            nc.vector.tensor_scalar_add(
                out=out_ap, in0=ps[:], scalar1=b_tiles[bname][:])
            yield

        def vext_quanta(kt):
            vps = att_psum.tile([128, 128], BF16, tag="fill", bufs=2,
                                name="vtr_ps")
            nc.tensor.transpose(vps[:], VT[:, 128 * kt:128 * (kt + 1)],
                                ident[:])
            for h in range(HL):
                nc.vector.tensor_copy(out=Vext[:, h, kt, 0:D],
                                      in_=vps[:, D * h:D * (h + 1)])
            yield

        def run_all(gen):
            for _ in gen:
                pass

        # ---- pre-phase: only the projections gated on the FIRST x^T
        # chunk-pair (b0 q-cols 0:1024); warm-drip dummies span the DMA ramp
        # so the HAM clock-gate stays open. Everything else weaves through
        # the attention ticks as deadline-ordered filler. ----
        bps = att_psum.tile([128, 3], F32, tag="fill", bufs=2, name="bps")
        nc.tensor.transpose(bps[:], b_row[:], ident32[0:3, 0:3])
        nc.vector.tensor_copy(out=b_sb[:], in_=bps[:])
        for j, bname in enumerate(("bq", "bk", "bv")):
            b_tiles[bname] = b_sb[:, j:j + 1]
        warmup(8, ident[:, 0:128], 8)
        run_all(proj_quanta("wk", "bk", KT, 0))
        warmup(2, ident[:, 0:128], 8)
        run_all(proj_quanta("wk", "bk", KT, 1))
        warmup(2, ident[:, 0:128], 8)
        run_all(proj_quanta("wq", "bq", QT, 0))
        run_all(proj_quanta("wq", "bq", QT, 1))
        for rb in (0, 1):
            run_all(proj_quanta("wv", "bv", VT, rb))
            for kt in range(4 * rb, 4 * rb + 4):
                run_all(vext_quanta(kt))
        warmup(0, ident[:, 0:128], 8, flush=True)

        # dummy collective #2: re-sync the CC queue before the attention phase
        sync2_in = dram.tile([1, 256], BF16)
        sync2_out = dram.tile([1, 256], BF16)
        nc.sync.dma_start(out=sync2_in[:], in_=VT[0:1, 1792:2048])
        nc.gpsimd.collective_compute(
            "AllReduce", mybir.AluOpType.add,
            replica_groups=[list(range(N_CORES))],
            ins=[sync2_in.opt()], outs=[sync2_out.opt()])

        # ---- attention: fine-grained tick pipeline ----
        a2a1_in = dram.tile([NG * HIDL, HB], BF16)
        a2a1_out = dram.tile([NG * HIDL, HB], BF16)
        a2a2_in = dram.tile([NG * HIDL, HB], BF16)
        a2a2_out = dram.tile([NG * HIDL, HB], BF16)
        ATn = big.tile([128, NG, QB], BF16)
        AT1 = big.tile([128, EC, HB], BF16)
        AT2 = big.tile([128, EC, HB], BF16)

        eb = {}
        pvT = {}

        def emit_scores(t):
            u, j = divmod(t, S128)
            b, s0a, s0b = UNIT_MAP[u]
            X = att_psum.tile([128, 2, QB], F32, tag="sc", bufs=2,
                              name="sc_ps")
            blk = (b * S + s0a) // HB
            for h in range(HL):
                hs = slice(64 * h, 64 * (h + 1))
                nc.tensor.matmul(
                    X[:, h, :],
                    KT[hs, b * S + 128 * j:b * S + 128 * (j + 1)],
                    QT[hs, blk:blk + 3:2, :],
                    start=True, stop=True)
            e = ebp.tile([128, 2, QB], BF16, tag="eb", bufs=6, name="eb")
            nc.scalar.activation(e[:], X[:], AF.Exp, scale=0.125)
            eb[t] = e

        def emit_pv(t):
            u, j = divmod(t, S128)
            b = UNIT_MAP[u][0]
            for h in range(HL):
                if j == 0:
                    pvT[(u, h)] = att_psum.tile(
                        [D + 1, QB], F32, tag=f"pv{h}", bufs=1,
                        name=f"pv{h}_ps")
                nc.tensor.matmul(pvT[(u, h)][:],
                                 Vext[:, h, b * S128 + j, :],
                                 eb[t][:, h, :],
                                 start=(j == 0), stop=(j == S128 - 1))
            del eb[t]

        def emit_norm(u):
            for h in range(HL):
                hs = slice(64 * h, 64 * (h + 1))
                pvsb = rp.tile([D + 1, QB], F32, tag=f"pvsb{h}", bufs=2,
                               name=f"pvsb{h}")
                nc.vector.tensor_copy(out=pvsb[:], in_=pvT[(u, h)][:])
                den = rp.tile([1, QB], F32, tag=f"den{h}", bufs=2,
                              name=f"den{h}")
                nc.vector.tensor_copy(out=den[:], in_=pvT[(u, h)][D:D + 1, :])
                r_row = rp.tile([1, QB], F32, tag=f"rr{h}", bufs=2,
                                name=f"rr{h}")
                nc.vector.reciprocal_approx_fast(r_row[:], den[:])
                r_sb = rp.tile([D, QB], F32, tag=f"rb{h}", bufs=2,
                               name=f"rb{h}")
                nc.gpsimd.partition_broadcast(r_sb[:], r_row[:])
                nc.vector.tensor_mul(
                    out=ATn[hs, u, :], in0=pvsb[0:D, :], in1=r_sb[:])
            # unit u's two 256-col halves are shards 2u', 2u'+1 of its A2A
            a_in = a2a1_in if u < 4 else a2a2_in
            ushard = 2 * (u % 4)
            for k in range(2):
                nc.sync.dma_start(
                    out=a_in[HIDL * (ushard + k):HIDL * (ushard + k + 1), :],
                    in_=ATn[:, u, HB * k:HB * (k + 1)])

        def oproj_quanta(qq, AT, o_rows):
            """Output projection for one 128-row block (quantum generator)."""
            o_sb = stage.tile([128, E], F32, tag="osb", bufs=2, name="osb")
            for e_c in range(E // QB):
                ps = att_psum.tile([128, QB], F32, tag="fill", bufs=2,
                                   name="op_ps")
                nc.tensor.matmul(ps[:], ones_st[:],
                                 bo_sb[:, QB * e_c:QB * (e_c + 1)],
                                 start=True, stop=False)
                for i in range(EC):
                    nc.tensor.matmul(ps[:], AT[:, i, 128 * qq:128 * (qq + 1)],
                                     wo_tiles[i][:, QB * e_c:QB * (e_c + 1)],
                                     start=False, stop=(i == EC - 1))
                    if i % 3 == 2:
                        yield
                nc.vector.tensor_copy(out=o_sb[:, QB * e_c:QB * (e_c + 1)],
                                      in_=ps[:])
                eng = nc.sync if e_c == 0 else nc.scalar
                eng.dma_start(
                    out=out_d[o_rows:o_rows + 128, QB * e_c:QB * (e_c + 1)],
                    in_=o_sb[:, QB * e_c:QB * (e_c + 1)])
                yield

        def filler_gen():
            # deadline-ordered: (rest of b0 for units 0-1) then b1 staged to
            # land just before units 2-3 consume each piece
            yield from proj_quanta("wk", "bk", KT, 2)
            yield from proj_quanta("wk", "bk", KT, 3)
            yield from proj_quanta("wv", "bv", VT, 2)
            for kt in range(8, 12):
                yield from vext_quanta(kt)
            yield from proj_quanta("wv", "bv", VT, 3)
            for kt in range(12, 16):
                yield from vext_quanta(kt)
            yield from proj_quanta("wq", "bq", QT, 2)
            yield from proj_quanta("wq", "bq", QT, 3)
            yield from proj_quanta("wk", "bk", KT, NQB)
            yield from proj_quanta("wq", "bq", QT, NQB)
            yield from proj_quanta("wq", "bq", QT, NQB + 1)
            yield from proj_quanta("wv", "bv", VT, NQB)
            for kt in range(16, 20):
                yield from vext_quanta(kt)
            yield from proj_quanta("wk", "bk", KT, NQB + 1)
            yield from proj_quanta("wv", "bv", VT, NQB + 1)
            for kt in range(20, 24):
                yield from vext_quanta(kt)
            yield from proj_quanta("wk", "bk", KT, NQB + 2)
            yield from proj_quanta("wv", "bv", VT, NQB + 2)
            for kt in range(24, 28):
                yield from vext_quanta(kt)
            yield from proj_quanta("wk", "bk", KT, NQB + 3)
            yield from proj_quanta("wq", "bq", QT, NQB + 2)
            yield from proj_quanta("wq", "bq", QT, NQB + 3)
            yield from proj_quanta("wv", "bv", VT, NQB + 3)
            for kt in range(28, 32):
                yield from vext_quanta(kt)

        def oproj1_gen():
            # first-half output projection; only consumed well after A2A#1
            # has landed (its matmuls would otherwise block the in-order PE
            # queue on the collective)
            yield from oproj_quanta(0, AT1, 0)
            yield from oproj_quanta(1, AT1, 128)

        fill = filler_gen()
        fill2 = oproj1_gen()
        fills_left = True
        fills2_left = True
        for t in range(NT + 2):
            if t < NT:
                emit_scores(t)
            if t >= 2:
                emit_pv(t - 2)
                u_done, j_done = divmod(t - 2, S128)
                if j_done == S128 - 1:
                    emit_norm(u_done)
                    if u_done == 3:
                        # first-half shards complete: launch hidden A2A#1.
                        # (Its unload DMA waits would block whichever engine
                        # queue carries them, so the unloads are emitted
                        # post-loop when the collective has long finished.)
                        nc.gpsimd.collective_compute(
                            "AllToAll", mybir.AluOpType.bypass,
                            replica_groups=[list(range(N_CORES))],
                            ins=[a2a1_in.opt()], outs=[a2a1_out.opt()])
            n_q = 3 if t < 16 else (2 if t < 56 else 1)
            for _ in range(n_q):
                if fills_left:
                    try:
                        next(fill)
                    except StopIteration:
                        fills_left = False

        nc.gpsimd.collective_compute(
            "AllToAll", mybir.AluOpType.bypass,
            replica_groups=[list(range(N_CORES))],
            ins=[a2a2_in.opt()], outs=[a2a2_out.opt()])
        for i in range(N_CORES):
            nc.sync.dma_start(out=AT1[:, i, :],
                              in_=a2a1_out[HIDL * i:HIDL * (i + 1), :])

        # drain leftover fillers; the first-half out projection is real PE
        # work that covers the A2A#2 flight (AT1 landed long ago)
        while fills_left:
            try:
                next(fill)
            except StopIteration:
                fills_left = False
        while fills2_left:
            try:
                next(fill2)
            except StopIteration:
                fills2_left = False
        warmup(8, ATn[:, NG - 1, 0:QB], 4, flush=True)
        for i in range(N_CORES):
            nc.sync.dma_start(out=AT2[:, i, :],
                              in_=a2a2_out[HIDL * i:HIDL * (i + 1), :])

        # ---- second-half out projection ----
        run_all(oproj_quanta(0, AT2, 256))
        run_all(oproj_quanta(1, AT2, 384))
        att_stack.close()

    nc.compile()
    return nc


def shard_inputs(x, Wq, bq, Wk, bk, Wv, bv, Wo, bo, N_CORES=8):
    """Host-side sharding: full fp32 inputs -> per-core in_maps."""
    import ml_dtypes
    bf16 = ml_dtypes.bfloat16
    B, S, E = x.shape
    R = B * S
    HIDL = E // N_CORES
    xt = np.ascontiguousarray(x.reshape(R, E).T).astype(bf16)
    wo = np.ascontiguousarray(Wo).astype(bf16)
    bo_b = np.ascontiguousarray(bo.reshape(1, E)).astype(bf16)
    EC = E // 128
    in_maps = []
    for c in range(N_CORES):
        cs = slice(HIDL * c, HIDL * (c + 1))
        # pack wq/wk/wv E-chunks side by side: rows are 6KB contiguous
        wqkv = np.empty((128, 3 * EC * HIDL), dtype=bf16)
        for wi, W in enumerate((Wq, Wk, Wv)):
            Wc = W[:, cs]
            for i in range(EC):
                blk = EC * wi + i
                wqkv[:, HIDL * blk:HIDL * (blk + 1)] = Wc[128 * i:128 * (i + 1), :]
        bqkv = np.stack([bq[cs], bk[cs], bv[cs]]).astype(np.float32)
        in_maps.append({
            "xt": xt,
            "wqkv": wqkv,
            "wo": wo,
            "bqkv": np.ascontiguousarray(bqkv),
            "bo": bo_b,
        })
    return in_maps


def kernel(x, Wq, bq, Wk, bk, Wv, bv, Wo, bo):
    from concourse.bass_utils import run_bass_kernel_spmd

    args = [np.asarray(a, dtype=np.float32) for a in
            (x, Wq, bq, Wk, bk, Wv, bv, Wo, bo)]
    if "nc" not in _CACHE:
        _CACHE["nc"] = build_kernel()
    nc = _CACHE["nc"]
    in_maps = shard_inputs(*args)
    res = run_bass_kernel_spmd(nc, in_maps, core_ids=list(range(8)))
    out = np.concatenate([res.results[i]["out"] for i in range(8)], axis=0)
    return out.reshape(2, 2048, 1024)
